# revision 9
# baseline (speedup 1.0000x reference)
"""BailingMoeV2.5 linear-attention layer on 8 Trainium2 NeuronCores.

Sharding: 2-way data parallel over batch x 4-way tensor parallel over heads
(4 heads per core). Each core computes qkv+gate projections for its heads,
qk-norm + partial RoPE, the chunked simple-GLA linear-attention scan, group
RMSNorm + sigmoid gating, and a partial output projection; the host sums the
4 partial outputs per batch.

Math notes:
- The chunked GLA scan is chunk-size invariant; we use C=128 (reference: 64).
- Per-chunk decay is folded into per-token scalars: qhat = q*exp(g*(i+1)),
  khat = k*exp(-g*(i+1)) so attT = khat^T qhat needs only a 0/1 causal mask
  and the inter/state terms reuse qhat / khat*exp(g*C) directly.
- Projections and attention run in fp32r (TF32, full PE rate at N>=512 /
  verified exact product of tf32-rounded operands); the final dense
  projection runs in bf16. PSUM accumulation is fp32 everywhere.
"""
import sys
sys.path.insert(0, '/opt/trn_rl_repo')
import math
import numpy as np
import ml_dtypes

import concourse.bass as bass
import concourse.bacc as bacc
import concourse.mybir as mybir
import concourse.tile as tile
from concourse.masks import make_identity
from concourse.bass_utils import run_bass_kernel_spmd

B, T, HID = 2, 4096, 2048
H, D = 16, 128
ROPE_DIM = 64
HALF = ROPE_DIM // 2
THETA = 10000.0
EPS = 1e-6
LAYER_IDX, N_LAYERS = 12, 32
C = 128                 # device chunk size
NT = T // C             # 32 token tiles per core
HL = 4                  # heads per core
NCORES = 8
KC = HID // 128         # 16 contraction chunks for qkv/gate
F32, F32R, BF16 = mybir.dt.float32, mybir.dt.float32r, mybir.dt.bfloat16
MULT, ADD = mybir.AluOpType.mult, mybir.AluOpType.add


def _tf32(x):
    b = np.ascontiguousarray(x, dtype=np.float32).view(np.uint32)
    b = (b + 0x1000 + ((b >> 13) & 1)) & np.uint32(0xFFFFE000)
    return b.view(np.float32)


def _slopes():
    start = 2.0 ** (-(2.0 ** -(math.log2(H) - 3.0)))
    s = np.array([start ** (i + 1) for i in range(H)], dtype=np.float64)
    scale = 1.0 - (LAYER_IDX - 1) / (N_LAYERS - 1) + 1e-5
    return -s * scale  # [H], negative per-step log-decay


def _bcast(handle, parts=128):
    ap = handle.ap()
    return bass.AP(tensor=ap.tensor, offset=ap.offset,
                   ap=[[0, parts]] + list(ap.ap))


def build_program():
    nc = bacc.Bacc()

    hsT = nc.dram_tensor("hsT", [HID, T], F32R, kind="ExternalInput")
    w_all = nc.dram_tensor("w_all", [HID, 2048], F32R, kind="ExternalInput")
    w_dT = nc.dram_tensor("w_dT", [512, 2048], BF16, kind="ExternalInput")
    cs_d = nc.dram_tensor("cs", [T, ROPE_DIM], F32, kind="ExternalInput")
    sn_d = nc.dram_tensor("sn", [T, ROPE_DIM], F32, kind="ExternalInput")
    qsc_d = nc.dram_tensor("qsc", [T, HL], F32, kind="ExternalInput")
    ksc_d = nc.dram_tensor("ksc", [T, HL], F32, kind="ExternalInput")
    lnq_d = nc.dram_tensor("lnq", [D], F32, kind="ExternalInput")
    lnk_d = nc.dram_tensor("lnk", [D], F32, kind="ExternalInput")
    gnw_d = nc.dram_tensor("gnw", [HL, D], F32, kind="ExternalInput")
    chd_d = nc.dram_tensor("chd", [HL], F32, kind="ExternalInput")
    msk_d = nc.dram_tensor("msk", [C, C], F32, kind="ExternalInput")
    out_d = nc.dram_tensor("out", [T, HID], F32, kind="ExternalOutput")

    with tile.TileContext(nc) as tc:
        with tc.tile_pool(name="consts", bufs=1) as cp, \
             tc.tile_pool(name="weights", bufs=1) as wp, \
             tc.tile_pool(name="state", bufs=1) as stp, \
             tc.tile_pool(name="hin", bufs=2) as hp, \
             tc.tile_pool(name="mid", bufs=2) as mp, \
             tc.tile_pool(name="ah", bufs=2) as ap_, \
             tc.tile_pool(name="ob", bufs=2) as obp, \
             tc.tile_pool(name="ps_big", bufs=2, space="PSUM") as psb, \
             tc.tile_pool(name="ps_small", bufs=3, space="PSUM") as pss, \
             tc.tile_pool(name="ps_dense", bufs=2, space="PSUM") as psd:

            # ---- constants / weights ----
            ident32 = cp.tile([128, 128], F32)
            make_identity(nc, ident32[:])
            ident_r = cp.tile([128, 128], F32R)
            nc.vector.tensor_copy(ident_r[:], ident32[:])
            ident_bf = cp.tile([128, 128], BF16)
            nc.vector.tensor_copy(ident_bf[:], ident32[:])

            maskT = cp.tile([C, C], F32)
            nc.sync.dma_start(out=maskT[:], in_=msk_d[:, :])
            lnq_bc = cp.tile([128, D], F32)
            nc.sync.dma_start(out=lnq_bc[:], in_=_bcast(lnq_d))
            lnk_bc = cp.tile([128, D], F32)
            nc.sync.dma_start(out=lnk_bc[:], in_=_bcast(lnk_d))
            gnw_bc = cp.tile([128, HL, D], F32)
            nc.sync.dma_start(out=gnw_bc[:], in_=_bcast(gnw_d))
            chd_bc = cp.tile([128, HL], F32)
            nc.sync.dma_start(out=chd_bc[:], in_=_bcast(chd_d))
            eps_t = cp.tile([128, 1], F32)
            nc.vector.memset(eps_t[:], EPS)

            w_all_sb = wp.tile([128, KC, 2048], F32R)
            nc.sync.dma_start(
                out=w_all_sb[:],
                in_=w_all.ap().rearrange("(kc kp) n -> kp kc n", kp=128))
            w_dT_sb = wp.tile([128, 4, 2048], BF16)
            nc.sync.dma_start(
                out=w_dT_sb[:],
                in_=w_dT.ap().rearrange("(kc kp) n -> kp kc n", kp=128))

            S_sb = stp.tile([128, HL, D], F32R)
            nc.vector.memset(S_sb[:].bitcast(F32), 0.0)

            hsT_r = hsT.ap().rearrange("(kc kp) t -> kp kc t", kp=128)

            for i in range(NT):
                tsl = slice(i * C, (i + 1) * C)
                ht = hp.tile([128, KC, C], F32R, tag="ht")
                nc.sync.dma_start(out=ht[:], in_=hsT_r[:, :, tsl])

                cs_t = mp.tile([C, ROPE_DIM], F32, tag="cs")
                nc.sync.dma_start(out=cs_t[:], in_=cs_d[tsl, :])
                sn_t = mp.tile([C, ROPE_DIM], F32, tag="sn")
                nc.sync.dma_start(out=sn_t[:], in_=sn_d[tsl, :])
                qsc_t = mp.tile([C, HL], F32, tag="qsc")
                nc.sync.dma_start(out=qsc_t[:], in_=qsc_d[tsl, :])
                ksc_t = mp.tile([C, HL], F32, tag="ksc")
                nc.sync.dma_start(out=ksc_t[:], in_=ksc_d[tsl, :])

                # ---- qkv + gate projections (token-major) ----
                def proj(nb):
                    ps = psb.tile([C, 512], F32, tag="qkvg")
                    for kc in range(KC):
                        nc.tensor.matmul(ps[:], ht[:, kc, :],
                                         w_all_sb[:, kc, nb * 512:(nb + 1) * 512],
                                         start=(kc == 0), stop=(kc == KC - 1))
                    return ps

                def norm_rope(ps, sc_all, ln_bc, out_r, scratch, pfx):
                    # per-head RMSNorm * ln_w * per-token scalar, then partial rope
                    ss = mp.tile([C, HL], F32, tag="ss" + pfx)
                    sq = mp.tile([C, D], F32, tag="scr")
                    for j in range(HL):
                        nc.scalar.activation(sq[:], ps[:, j * D:(j + 1) * D],
                                             mybir.ActivationFunctionType.Square,
                                             accum_out=ss[:, j:j + 1])
                    sc = mp.tile([C, HL], F32, tag="sc" + pfx)
                    nc.scalar.activation(sc[:], ss[:],
                                         mybir.ActivationFunctionType.Sqrt,
                                         bias=eps_t[:], scale=1.0 / D)
                    nc.vector.reciprocal(sc[:], sc[:])
                    nc.vector.tensor_mul(sc[:], sc[:], sc_all[:])
                    for j in range(HL):
                        scj = sc[:, j:j + 1]
                        base = j * D
                        # rope half (dims 0:64) -> f32 scratch; rest direct
                        nc.vector.scalar_tensor_tensor(
                            out=scratch[:, j, :], in0=ps[:, base:base + ROPE_DIM],
                            scalar=scj, in1=ln_bc[:, 0:ROPE_DIM], op0=MULT, op1=MULT)
                        nc.vector.scalar_tensor_tensor(
                            out=out_r[:, j, ROPE_DIM:D],
                            in0=ps[:, base + ROPE_DIM:base + D],
                            scalar=scj, in1=ln_bc[:, ROPE_DIM:D], op0=MULT, op1=MULT)
                        x0 = scratch[:, j, 0:HALF]
                        x1 = scratch[:, j, HALF:ROPE_DIM]
                        r0 = mp.tile([C, HALF], F32, tag="r0")
                        m1 = mp.tile([C, HALF], F32, tag="m1")
                        nc.vector.tensor_mul(r0[:], x0, cs_t[:, 0:HALF])
                        nc.vector.tensor_mul(m1[:], x1, sn_t[:, 0:HALF])
                        nc.vector.scalar_tensor_tensor(
                            out=out_r[:, j, 0:HALF], in0=m1[:], scalar=-1.0,
                            in1=r0[:], op0=MULT, op1=ADD)
                        r1 = mp.tile([C, HALF], F32, tag="r1")
                        m0 = mp.tile([C, HALF], F32, tag="m0")
                        nc.vector.tensor_mul(r1[:], x1, cs_t[:, HALF:ROPE_DIM])
                        nc.vector.tensor_mul(m0[:], x0, sn_t[:, HALF:ROPE_DIM])
                        nc.vector.tensor_add(
                            out_r[:, j, HALF:ROPE_DIM], r1[:], m0[:])

                qh = mp.tile([C, HL, D], F32R, tag="qh")
                qa = mp.tile([C, HL, ROPE_DIM], F32, tag="qa")
                ps_q = proj(0)
                norm_rope(ps_q, qsc_t, lnq_bc, qh, qa, "q")

                kh = mp.tile([C, HL, D], F32R, tag="kh")
                ka = mp.tile([C, HL, ROPE_DIM], F32, tag="ka")
                ps_k = proj(1)
                norm_rope(ps_k, ksc_t, lnk_bc, kh, ka, "k")

                ps_v = proj(2)
                v_r = mp.tile([C, HL, D], F32R, tag="v_r")
                nc.scalar.copy(v_r[:], ps_v[:])

                ps_g = proj(3)
                g_sb = mp.tile([C, HL, D], F32, tag="g_sb")
                nc.scalar.activation(g_sb[:], ps_g[:],
                                     mybir.ActivationFunctionType.Sigmoid)

                # ---- attention scan + gating, per head ----
                o_sb = mp.tile([C, HL, D], F32, tag="o_sb")
                oss = mp.tile([C, HL], F32, tag="oss")
                og_bf = mp.tile([C, HL, D], BF16, tag="og_bf")
                ogT = mp.tile([128, HL, C], BF16, tag="ogT")
                osq = mp.tile([C, D], F32, tag="scr")
                for j in range(HL):
                    chj = chd_bc[:, j:j + 1]
                    # feature-major qhat/khat via PE transpose
                    pt_q = pss.tile([128, C], F32R, tag="sp")
                    nc.tensor.transpose(pt_q[:], qh[:, j, :], ident_r[:])
                    qT = ap_.tile([128, C], F32R, tag="qT")
                    nc.vector.tensor_copy(qT[:], pt_q[:])
                    pt_k = pss.tile([128, C], F32R, tag="sp")
                    nc.tensor.transpose(pt_k[:], kh[:, j, :], ident_r[:])
                    kT = ap_.tile([128, C], F32R, tag="kT")
                    nc.vector.tensor_copy(kT[:], pt_k[:])

                    att_ps = pss.tile([C, C], F32, tag="sp")
                    nc.tensor.matmul(att_ps[:], kT[:], qT[:])
                    att_r = ap_.tile([C, C], F32R, tag="att_r")
                    nc.vector.tensor_mul(att_r[:], att_ps[:], maskT[:])

                    o_ps = pss.tile([C, D], F32, tag="sp")
                    nc.tensor.matmul(o_ps[:], att_r[:], v_r[:, j, :],
                                     start=True, stop=False)
                    nc.tensor.matmul(o_ps[:], qT[:], S_sb[:, j, :],
                                     start=False, stop=True)

                    kch = ap_.tile([C, D], F32R, tag="kch")
                    nc.vector.tensor_scalar_mul(kch[:], kh[:, j, :], chj)
                    sd_ps = pss.tile([128, D], F32, tag="sp")
                    nc.tensor.matmul(sd_ps[:], kch[:], v_r[:, j, :])
                    nc.vector.scalar_tensor_tensor(
                        out=S_sb[:, j, :], in0=S_sb[:, j, :], scalar=chj,
                        in1=sd_ps[:], op0=MULT, op1=ADD)

                    nc.scalar.copy(o_sb[:, j, :], o_ps[:])
                    nc.scalar.activation(osq[:], o_sb[:, j, :],
                                         mybir.ActivationFunctionType.Square,
                                         accum_out=oss[:, j:j + 1])

                ro = mp.tile([C, HL], F32, tag="ro")
                nc.scalar.activation(ro[:], oss[:],
                                     mybir.ActivationFunctionType.Sqrt,
                                     bias=eps_t[:], scale=1.0 / D)
                nc.vector.reciprocal(ro[:], ro[:])
                ogs = mp.tile([C, D], F32, tag="scr")
                for j in range(HL):
                    nc.vector.scalar_tensor_tensor(
                        out=ogs[:], in0=o_sb[:, j, :], scalar=ro[:, j:j + 1],
                        in1=gnw_bc[:, j, :], op0=MULT, op1=MULT)
                    nc.vector.tensor_mul(og_bf[:, j, :], ogs[:], g_sb[:, j, :])
                    pt_o = pss.tile([128, C], BF16, tag="sp")
                    nc.tensor.transpose(pt_o[:], og_bf[:, j, :], ident_bf[:])
                    nc.vector.tensor_copy(ogT[:, j, :], pt_o[:])

                # ---- dense partial projection ----
                for nb in range(4):
                    dps = psd.tile([C, 512], F32, tag="dense")
                    for kc in range(4):
                        nc.tensor.matmul(dps[:], ogT[:, kc, :],
                                         w_dT_sb[:, kc, nb * 512:(nb + 1) * 512],
                                         start=(kc == 0), stop=(kc == 3))
                    ob = obp.tile([C, 512], F32, tag="ob")
                    nc.scalar.copy(ob[:], dps[:])
                    nc.sync.dma_start(out=out_d[tsl, nb * 512:(nb + 1) * 512],
                                      in_=ob[:])

    nc.finalize()
    return nc


_PROGRAM = None


def prepare_in_maps(hidden_states, w_qkv, q_ln_w, k_ln_w, g_norm_w, w_g_proj,
                    w_dense, position_ids):
    hidden_states = np.asarray(hidden_states, dtype=np.float32)
    w_qkv = np.asarray(w_qkv, dtype=np.float32)
    q_ln_w = np.asarray(q_ln_w, dtype=np.float32)
    k_ln_w = np.asarray(k_ln_w, dtype=np.float32)
    g_norm_w = np.asarray(g_norm_w, dtype=np.float32)
    w_g_proj = np.asarray(w_g_proj, dtype=np.float32)
    w_dense = np.asarray(w_dense, dtype=np.float32)
    position_ids = np.asarray(position_ids, dtype=np.int32)

    g = _slopes()  # [H] float64

    # rope tables per batch
    inv_freq = 1.0 / (THETA ** (np.arange(0, ROPE_DIM, 2, dtype=np.float32)
                                / ROPE_DIM))
    cs_b, sn_b = [], []
    for b in range(B):
        freqs = position_ids[b].astype(np.float32)[:, None] * inv_freq[None, :]
        emb = np.concatenate([freqs, freqs], axis=-1)
        cs_b.append(np.cos(emb).astype(np.float32))
        sn_b.append(np.sin(emb).astype(np.float32))

    msk = np.tril(np.ones((C, C), dtype=np.float32)).T.copy()  # maskT[e,c]=c>=e
    ii = (np.arange(T) % C).astype(np.float64) + 1.0

    in_maps = []
    for c in range(NCORES):
        b, hg = c // 4, c % 4
        heads = [hg * HL + j for j in range(HL)]

        hsT = _tf32(hidden_states[b].T)

        rows = lambda w, base: np.concatenate(
            [w[base + h * D: base + (h + 1) * D] for h in heads], axis=0)
        w_all = np.concatenate([
            rows(w_qkv, 0), rows(w_qkv, H * D), rows(w_qkv, 2 * H * D),
            rows(w_g_proj, 0)], axis=0)                 # [2048, HID]
        w_all_T = _tf32(w_all.T)

        cols = np.concatenate([np.arange(h * D, (h + 1) * D) for h in heads])
        w_dT = np.ascontiguousarray(w_dense[:, cols].T).astype(ml_dtypes.bfloat16)

        gh = g[heads]                                    # [HL]
        qsc = (D ** -0.5) * np.exp(gh[None, :] * ii[:, None])
        ksc = np.exp(-gh[None, :] * ii[:, None])
        chd = np.exp(gh * C)

        in_maps.append({
            "hsT": hsT,
            "w_all": w_all_T,
            "w_dT": w_dT,
            "cs": cs_b[b], "sn": sn_b[b],
            "qsc": qsc.astype(np.float32), "ksc": ksc.astype(np.float32),
            "lnq": q_ln_w, "lnk": k_ln_w,
            "gnw": np.ascontiguousarray(g_norm_w.reshape(H, D)[heads]),
            "chd": chd.astype(np.float32),
            "msk": msk,
        })
    return in_maps


def kernel(hidden_states, w_qkv, q_ln_w, k_ln_w, g_norm_w, w_g_proj, w_dense,
           position_ids):
    global _PROGRAM
    in_maps = prepare_in_maps(hidden_states, w_qkv, q_ln_w, k_ln_w, g_norm_w,
                              w_g_proj, w_dense, position_ids)
    if _PROGRAM is None:
        _PROGRAM = build_program()
    res = run_bass_kernel_spmd(_PROGRAM, in_maps, list(range(NCORES)))

    out = np.zeros((B, T, HID), dtype=np.float32)
    for c in range(NCORES):
        out[c // 4] += res.results[c]["out"]
    return out


# revision 11
# speedup vs baseline: 1.2790x; 1.2790x over previous
"""BailingMoeV2.5 linear-attention layer on 8 Trainium2 NeuronCores.

Sharding: 2-way data parallel over batch x 4-way tensor parallel over heads
(4 heads per core). Each core computes qkv+gate projections for its heads,
qk-norm + partial RoPE, the chunked simple-GLA linear-attention scan, group
RMSNorm + sigmoid gating, and a partial output projection; the host sums the
4 partial outputs per batch.

Math notes:
- The chunked GLA scan is chunk-size invariant; we use C=128 (reference: 64).
- Per-chunk decay is folded into per-token scalars: qhat = q*exp(g*(i+1)),
  khat = k*exp(-g*(i+1)) so attT = khat^T qhat needs only a 0/1 causal mask
  and the inter/state terms reuse qhat / khat*exp(g*C) directly.
- Matmuls run in bf16 with fp32 PSUM accumulation; the recurrent state is
  kept in fp32r (tf32) with a bf16 shadow for the inter-chunk matmul read.
"""
import sys
sys.path.insert(0, '/opt/trn_rl_repo')
import math
import numpy as np
import ml_dtypes

import concourse.bass as bass
import concourse.bacc as bacc
import concourse.mybir as mybir
import concourse.tile as tile
from concourse.masks import make_identity
from concourse.bass_utils import run_bass_kernel_spmd

B, T, HID = 2, 4096, 2048
H, D = 16, 128
ROPE_DIM = 64
HALF = ROPE_DIM // 2
THETA = 10000.0
EPS = 1e-6
LAYER_IDX, N_LAYERS = 12, 32
C = 128                 # device chunk size
NT = T // C             # 32 token tiles per core
HL = 4                  # heads per core
NCORES = 8
KC = HID // 128         # 16 contraction chunks for qkv/gate
F32, F32R, BF16 = mybir.dt.float32, mybir.dt.float32r, mybir.dt.bfloat16
MULT, ADD = mybir.AluOpType.mult, mybir.AluOpType.add
SQUARE = mybir.ActivationFunctionType.Square
SQRT = mybir.ActivationFunctionType.Sqrt
SIGMOID = mybir.ActivationFunctionType.Sigmoid


def _slopes():
    start = 2.0 ** (-(2.0 ** -(math.log2(H) - 3.0)))
    s = np.array([start ** (i + 1) for i in range(H)], dtype=np.float64)
    scale = 1.0 - (LAYER_IDX - 1) / (N_LAYERS - 1) + 1e-5
    return -s * scale  # [H], negative per-step log-decay


def _bcast(handle, parts=128):
    ap = handle.ap()
    return bass.AP(tensor=ap.tensor, offset=ap.offset,
                   ap=[[0, parts]] + list(ap.ap))


def _bcast_mid(ap2d, n):
    # [P, W] -> [P, n, W] with stride-0 middle dim
    return bass.AP(tensor=ap2d.tensor, offset=ap2d.offset,
                   ap=[list(ap2d.ap[0]), [0, n], list(ap2d.ap[1])])


def build_program():
    nc = bacc.Bacc()

    hsT = nc.dram_tensor("hsT", [HID, T], BF16, kind="ExternalInput")
    w_all = nc.dram_tensor("w_all", [HID, 2048], BF16, kind="ExternalInput")
    w_dT = nc.dram_tensor("w_dT", [512, 2048], BF16, kind="ExternalInput")
    cs_d = nc.dram_tensor("cs", [T, ROPE_DIM], F32, kind="ExternalInput")
    sn_d = nc.dram_tensor("sn", [T, ROPE_DIM], F32, kind="ExternalInput")
    qsc_d = nc.dram_tensor("qsc", [T, HL], F32, kind="ExternalInput")
    ksc_d = nc.dram_tensor("ksc", [T, HL], F32, kind="ExternalInput")
    lnq_d = nc.dram_tensor("lnq", [D], F32, kind="ExternalInput")
    lnk_d = nc.dram_tensor("lnk", [D], F32, kind="ExternalInput")
    gnw_d = nc.dram_tensor("gnw", [HL, D], F32, kind="ExternalInput")
    chd_d = nc.dram_tensor("chd", [HL], F32, kind="ExternalInput")
    msk_d = nc.dram_tensor("msk", [C, C], F32, kind="ExternalInput")
    out_d = nc.dram_tensor("out", [T, HID], F32, kind="ExternalOutput")

    with tile.TileContext(nc) as tc:
        with tc.tile_pool(name="consts", bufs=1) as cp, \
             tc.tile_pool(name="weights", bufs=1) as wp, \
             tc.tile_pool(name="state", bufs=1) as stp, \
             tc.tile_pool(name="hin", bufs=3) as hp, \
             tc.tile_pool(name="mid", bufs=2) as mp, \
             tc.tile_pool(name="ah", bufs=2) as ap_, \
             tc.tile_pool(name="ob", bufs=3) as obp, \
             tc.tile_pool(name="ps_big", bufs=2, space="PSUM") as psb, \
             tc.tile_pool(name="ps_small", bufs=3, space="PSUM") as pss, \
             tc.tile_pool(name="ps_dense", bufs=2, space="PSUM") as psd:

            # ---- constants / weights ----
            ident32 = cp.tile([128, 128], F32)
            make_identity(nc, ident32[:])
            ident_bf = cp.tile([128, 128], BF16)
            nc.vector.tensor_copy(ident_bf[:], ident32[:])

            maskT = cp.tile([C, C], F32)
            nc.sync.dma_start(out=maskT[:], in_=msk_d[:, :])
            lnq_bc = cp.tile([128, D], F32)
            nc.sync.dma_start(out=lnq_bc[:], in_=_bcast(lnq_d))
            lnk_bc = cp.tile([128, D], F32)
            nc.sync.dma_start(out=lnk_bc[:], in_=_bcast(lnk_d))
            gnw_bc = cp.tile([128, HL, D], F32)
            nc.sync.dma_start(out=gnw_bc[:], in_=_bcast(gnw_d))
            chd_bc = cp.tile([128, HL], F32)
            nc.sync.dma_start(out=chd_bc[:], in_=_bcast(chd_d))
            eps_t = cp.tile([128, 1], F32)
            nc.vector.memset(eps_t[:], EPS)

            w_all_sb = wp.tile([128, KC, 2048], BF16)
            w_all_r = w_all.ap().rearrange("(kc kp) n -> kp kc n", kp=128)
            for kc in range(KC):
                nc.sync.dma_start(out=w_all_sb[:, kc, :], in_=w_all_r[:, kc, :])
            w_dT_sb = wp.tile([128, 4, 2048], BF16)
            w_dT_r = w_dT.ap().rearrange("(kc kp) n -> kp kc n", kp=128)
            for kc in range(4):
                nc.sync.dma_start(out=w_dT_sb[:, kc, :], in_=w_dT_r[:, kc, :])

            S_r = stp.tile([128, HL, D], F32R)
            nc.vector.memset(S_r[:].bitcast(F32), 0.0)
            S_bf = stp.tile([128, HL, D], BF16)
            nc.vector.memset(S_bf[:].bitcast(mybir.dt.uint16), 0)

            hsT_r = hsT.ap().rearrange("(kc kp) t -> kp kc t", kp=128)

            for i in range(NT):
                tsl = slice(i * C, (i + 1) * C)
                ht = hp.tile([128, KC, C], BF16, tag="ht")
                nc.sync.dma_start(out=ht[:], in_=hsT_r[:, :, tsl])

                cs_t = mp.tile([C, ROPE_DIM], F32, tag="cs")
                nc.sync.dma_start(out=cs_t[:], in_=cs_d[tsl, :])
                sn_t = mp.tile([C, ROPE_DIM], F32, tag="sn")
                nc.sync.dma_start(out=sn_t[:], in_=sn_d[tsl, :])
                qsc_t = mp.tile([C, HL], F32, tag="qsc")
                nc.sync.dma_start(out=qsc_t[:], in_=qsc_d[tsl, :])
                ksc_t = mp.tile([C, HL], F32, tag="ksc")
                nc.sync.dma_start(out=ksc_t[:], in_=ksc_d[tsl, :])

                # ---- qkv + gate projections (token-major) ----
                def proj(nb):
                    ps = psb.tile([C, 512], F32, tag="qkvg")
                    for kc in range(KC):
                        nc.tensor.matmul(ps[:], ht[:, kc, :],
                                         w_all_sb[:, kc, nb * 512:(nb + 1) * 512],
                                         start=(kc == 0), stop=(kc == KC - 1))
                    return ps

                def norm_rope(ps, sc_all, ln_bc, out_r, scratch, pfx):
                    # per-head RMSNorm * ln_w * per-token scalar, then rope
                    ss = mp.tile([C, HL], F32, tag="ss" + pfx)
                    sq = mp.tile([C, D], F32, tag="scr")
                    for j in range(HL):
                        nc.scalar.activation(sq[:], ps[:, j * D:(j + 1) * D],
                                             SQUARE, accum_out=ss[:, j:j + 1])
                    sc = mp.tile([C, HL], F32, tag="sc" + pfx)
                    nc.scalar.activation(sc[:], ss[:], SQRT,
                                         bias=eps_t[:], scale=1.0 / D)
                    nc.vector.reciprocal(sc[:], sc[:])
                    nc.vector.tensor_mul(sc[:], sc[:], sc_all[:])
                    for j in range(HL):
                        scj = sc[:, j:j + 1]
                        base = j * D
                        nc.vector.scalar_tensor_tensor(
                            out=scratch[:, j, :], in0=ps[:, base:base + ROPE_DIM],
                            scalar=scj, in1=ln_bc[:, 0:ROPE_DIM], op0=MULT, op1=MULT)
                        nc.vector.scalar_tensor_tensor(
                            out=out_r[:, j, ROPE_DIM:D],
                            in0=ps[:, base + ROPE_DIM:base + D],
                            scalar=scj, in1=ln_bc[:, ROPE_DIM:D], op0=MULT, op1=MULT)
                    # batched partial rope across heads (stride-0 cos/sin)
                    x0 = scratch[:, :, 0:HALF]
                    x1 = scratch[:, :, HALF:ROPE_DIM]
                    cs0 = _bcast_mid(cs_t[:, 0:HALF], HL)
                    cs1 = _bcast_mid(cs_t[:, HALF:ROPE_DIM], HL)
                    sn0 = _bcast_mid(sn_t[:, 0:HALF], HL)
                    sn1 = _bcast_mid(sn_t[:, HALF:ROPE_DIM], HL)
                    r0 = mp.tile([C, HL, HALF], F32, tag="r0")
                    m1 = mp.tile([C, HL, HALF], F32, tag="m1")
                    nc.vector.tensor_mul(r0[:], x0, cs0)
                    nc.vector.tensor_mul(m1[:], x1, sn0)
                    nc.vector.scalar_tensor_tensor(
                        out=out_r[:, :, 0:HALF], in0=m1[:], scalar=-1.0,
                        in1=r0[:], op0=MULT, op1=ADD)
                    r1 = mp.tile([C, HL, HALF], F32, tag="r1")
                    m0 = mp.tile([C, HL, HALF], F32, tag="m0")
                    nc.vector.tensor_mul(r1[:], x1, cs1)
                    nc.vector.tensor_mul(m0[:], x0, sn1)
                    nc.vector.tensor_add(out_r[:, :, HALF:ROPE_DIM], r1[:], m0[:])

                qh = mp.tile([C, HL, D], BF16, tag="qh")
                qa = mp.tile([C, HL, ROPE_DIM], F32, tag="qa")
                ps_q = proj(0)
                norm_rope(ps_q, qsc_t, lnq_bc, qh, qa, "q")

                kh = mp.tile([C, HL, D], BF16, tag="kh")
                ka = mp.tile([C, HL, ROPE_DIM], F32, tag="ka")
                ps_k = proj(1)
                norm_rope(ps_k, ksc_t, lnk_bc, kh, ka, "k")

                ps_v = proj(2)
                v_r = mp.tile([C, HL, D], BF16, tag="v_r")
                nc.scalar.copy(v_r[:], ps_v[:])

                ps_g = proj(3)
                g_sb = mp.tile([C, HL, D], F32, tag="g_sb")
                nc.scalar.activation(g_sb[:], ps_g[:], SIGMOID)

                # ---- attention scan: phase 1, feature-major q/k ----
                qT = [None] * HL
                kT = [None] * HL
                for j in range(HL):
                    pt_q = pss.tile([128, C], BF16, tag="sp")
                    nc.tensor.transpose(pt_q[:], qh[:, j, :], ident_bf[:])
                    qT[j] = ap_.tile([128, C], BF16, tag=f"qT{j}", name=f"qT{j}")
                    nc.vector.tensor_copy(qT[j][:], pt_q[:])
                    pt_k = pss.tile([128, C], BF16, tag="sp")
                    nc.tensor.transpose(pt_k[:], kh[:, j, :], ident_bf[:])
                    kT[j] = ap_.tile([128, C], BF16, tag=f"kT{j}", name=f"kT{j}")
                    nc.vector.tensor_copy(kT[j][:], pt_k[:])

                # phase 2: intra-chunk attention scores + decay-scaled k
                att = [None] * HL
                kch = [None] * HL
                for j in range(HL):
                    att_ps = pss.tile([C, C], F32, tag="sp")
                    nc.tensor.matmul(att_ps[:], kT[j][:], qT[j][:])
                    att[j] = ap_.tile([C, C], BF16, tag=f"att{j}", name=f"att{j}")
                    nc.vector.tensor_mul(att[j][:], att_ps[:], maskT[:])
                    kch[j] = ap_.tile([C, D], BF16, tag=f"kch{j}", name=f"kch{j}")
                    nc.vector.tensor_scalar_mul(kch[j][:], kh[:, j, :],
                                                chd_bc[:, j:j + 1])

                # phase 3: output + state update
                o_sb = mp.tile([C, HL, D], F32, tag="o_sb")
                oss = mp.tile([C, HL], F32, tag="oss")
                osq = mp.tile([C, D], F32, tag="scr")
                for j in range(HL):
                    chj = chd_bc[:, j:j + 1]
                    o_ps = pss.tile([C, D], F32, tag="sp")
                    nc.tensor.matmul(o_ps[:], att[j][:], v_r[:, j, :],
                                     start=True, stop=False)
                    nc.tensor.matmul(o_ps[:], qT[j][:], S_bf[:, j, :],
                                     start=False, stop=True)
                    sd_ps = pss.tile([128, D], F32, tag="sp")
                    nc.tensor.matmul(sd_ps[:], kch[j][:], v_r[:, j, :])
                    nc.vector.scalar_tensor_tensor(
                        out=S_r[:, j, :], in0=S_r[:, j, :], scalar=chj,
                        in1=sd_ps[:], op0=MULT, op1=ADD)
                    nc.vector.tensor_copy(S_bf[:, j, :], S_r[:, j, :])
                    nc.scalar.copy(o_sb[:, j, :], o_ps[:])
                    nc.scalar.activation(osq[:], o_sb[:, j, :], SQUARE,
                                         accum_out=oss[:, j:j + 1])

                # ---- group-norm + sigmoid gate, transpose for dense ----
                ro = mp.tile([C, HL], F32, tag="ro")
                nc.scalar.activation(ro[:], oss[:], SQRT,
                                     bias=eps_t[:], scale=1.0 / D)
                nc.vector.reciprocal(ro[:], ro[:])
                og_bf = mp.tile([C, HL, D], BF16, tag="og_bf")
                ogT = mp.tile([128, HL, C], BF16, tag="ogT")
                ogs = mp.tile([C, D], F32, tag="scr")
                for j in range(HL):
                    nc.vector.scalar_tensor_tensor(
                        out=ogs[:], in0=o_sb[:, j, :], scalar=ro[:, j:j + 1],
                        in1=gnw_bc[:, j, :], op0=MULT, op1=MULT)
                    nc.vector.tensor_mul(og_bf[:, j, :], ogs[:], g_sb[:, j, :])
                    pt_o = pss.tile([128, C], BF16, tag="sp")
                    nc.tensor.transpose(pt_o[:], og_bf[:, j, :], ident_bf[:])
                    nc.vector.tensor_copy(ogT[:, j, :], pt_o[:])

                # ---- dense partial projection ----
                for nb in range(4):
                    dps = psd.tile([C, 512], F32, tag="dense")
                    for kc in range(4):
                        nc.tensor.matmul(dps[:], ogT[:, kc, :],
                                         w_dT_sb[:, kc, nb * 512:(nb + 1) * 512],
                                         start=(kc == 0), stop=(kc == 3))
                    ob = obp.tile([C, 512], F32, tag="ob")
                    nc.scalar.copy(ob[:], dps[:])
                    nc.sync.dma_start(out=out_d[tsl, nb * 512:(nb + 1) * 512],
                                      in_=ob[:])

    nc.finalize()
    return nc


_PROGRAM = None


def prepare_in_maps(hidden_states, w_qkv, q_ln_w, k_ln_w, g_norm_w, w_g_proj,
                    w_dense, position_ids):
    hidden_states = np.asarray(hidden_states, dtype=np.float32)
    w_qkv = np.asarray(w_qkv, dtype=np.float32)
    q_ln_w = np.asarray(q_ln_w, dtype=np.float32)
    k_ln_w = np.asarray(k_ln_w, dtype=np.float32)
    g_norm_w = np.asarray(g_norm_w, dtype=np.float32)
    w_g_proj = np.asarray(w_g_proj, dtype=np.float32)
    w_dense = np.asarray(w_dense, dtype=np.float32)
    position_ids = np.asarray(position_ids, dtype=np.int32)

    g = _slopes()  # [H] float64

    inv_freq = 1.0 / (THETA ** (np.arange(0, ROPE_DIM, 2, dtype=np.float32)
                                / ROPE_DIM))
    cs_b, sn_b = [], []
    for b in range(B):
        freqs = position_ids[b].astype(np.float32)[:, None] * inv_freq[None, :]
        emb = np.concatenate([freqs, freqs], axis=-1)
        cs_b.append(np.cos(emb).astype(np.float32))
        sn_b.append(np.sin(emb).astype(np.float32))

    msk = np.tril(np.ones((C, C), dtype=np.float32)).T.copy()  # maskT[e,c]=c>=e
    ii = (np.arange(T) % C).astype(np.float64) + 1.0

    in_maps = []
    for c in range(NCORES):
        b, hg = c // 4, c % 4
        heads = [hg * HL + j for j in range(HL)]

        hsT = np.ascontiguousarray(hidden_states[b].T).astype(ml_dtypes.bfloat16)

        rows = lambda w, base: np.concatenate(
            [w[base + h * D: base + (h + 1) * D] for h in heads], axis=0)
        w_all = np.concatenate([
            rows(w_qkv, 0), rows(w_qkv, H * D), rows(w_qkv, 2 * H * D),
            rows(w_g_proj, 0)], axis=0)                 # [2048, HID]
        w_all_T = np.ascontiguousarray(w_all.T).astype(ml_dtypes.bfloat16)

        cols = np.concatenate([np.arange(h * D, (h + 1) * D) for h in heads])
        w_dT = np.ascontiguousarray(w_dense[:, cols].T).astype(ml_dtypes.bfloat16)

        gh = g[heads]                                    # [HL]
        qsc = (D ** -0.5) * np.exp(gh[None, :] * ii[:, None])
        ksc = np.exp(-gh[None, :] * ii[:, None])
        chd = np.exp(gh * C)

        in_maps.append({
            "hsT": hsT,
            "w_all": w_all_T,
            "w_dT": w_dT,
            "cs": cs_b[b], "sn": sn_b[b],
            "qsc": qsc.astype(np.float32), "ksc": ksc.astype(np.float32),
            "lnq": q_ln_w, "lnk": k_ln_w,
            "gnw": np.ascontiguousarray(g_norm_w.reshape(H, D)[heads]),
            "chd": chd.astype(np.float32),
            "msk": msk,
        })
    return in_maps


def kernel(hidden_states, w_qkv, q_ln_w, k_ln_w, g_norm_w, w_g_proj, w_dense,
           position_ids):
    global _PROGRAM
    in_maps = prepare_in_maps(hidden_states, w_qkv, q_ln_w, k_ln_w, g_norm_w,
                              w_g_proj, w_dense, position_ids)
    if _PROGRAM is None:
        _PROGRAM = build_program()
    res = run_bass_kernel_spmd(_PROGRAM, in_maps, list(range(NCORES)))

    out = np.zeros((B, T, HID), dtype=np.float32)
    for c in range(NCORES):
        out[c // 4] += res.results[c]["out"]
    return out
